# revision 1
# baseline (speedup 1.0000x reference)
"""BinaryLinear kernel for Trainium2, 8 NeuronCores.

y = x @ sign(W)^T + bias
  x: (8, 1024, 4096) f32, W: (4096, 4096) f32, bias: (4096,) f32
  y: (8, 1024, 4096) f32

Strategy: data-parallel over the batch dim (8 batches -> 8 cores).
Each core computes y_c[1024, 4096] = x_c @ sign(W)^T + bias as a bf16
matmul (sign(W) is exactly representable in bf16) with fp32 PSUM
accumulation. Host-side we only reshape/cast: wt = sign(W).T and
xt = x_c.T so the contraction dim lands on SBUF partitions with
contiguous DMA.
"""

import numpy as np
import ml_dtypes

import concourse.bass as bass
import concourse.tile as tile
from concourse import bacc, mybir
from concourse.bass_utils import run_bass_kernel_spmd

# Problem shapes (hardcoded per contract)
B, S, DIN, DOUT = 8, 1024, 4096, 4096
P = 128            # SBUF partitions / contraction tile
KT = DIN // P      # 32 contraction tiles
MT = S // P        # 8 row tiles of output (s dim)
NF = 512           # matmul moving free dim / PSUM bank width (fp32)
NB = DOUT // NF    # 8 column blocks of output (o dim)

N_CORES = 8


def build_nc():
    nc = bacc.Bacc("TRN2", target_bir_lowering=False, debug=False,
                   num_devices=N_CORES)
    xt = nc.dram_tensor("xt", [DIN, S], mybir.dt.bfloat16, kind="ExternalInput")
    wt = nc.dram_tensor("wt", [DIN, DOUT], mybir.dt.bfloat16, kind="ExternalInput")
    bias = nc.dram_tensor("bias", [P, DOUT], mybir.dt.float32, kind="ExternalInput")
    y = nc.dram_tensor("y", [S, DOUT], mybir.dt.float32, kind="ExternalOutput")

    xt_r = xt.ap().rearrange("(k p) s -> p k s", p=P)     # [128, 32, 1024]
    wt_r = wt.ap().rearrange("(k p) o -> p k o", p=P)     # [128, 32, 4096]
    y_ap = y.ap()
    bias_ap = bias.ap()

    with tile.TileContext(nc) as tc:
        with (
            tc.tile_pool(name="xpool", bufs=1) as xpool,
            tc.tile_pool(name="bpool", bufs=1) as bpool,
            tc.tile_pool(name="wpool", bufs=2) as wpool,
            tc.tile_pool(name="opool", bufs=4) as opool,
            tc.tile_pool(name="psum", bufs=4, space="PSUM") as psum,
        ):
            xt_sb = xpool.tile([P, KT, S], mybir.dt.bfloat16)
            nc.sync.dma_start(xt_sb[:], xt_r[:])

            bias_sb = bpool.tile([P, DOUT], mybir.dt.float32)
            nc.sync.dma_start(bias_sb[:], bias_ap[:])

            for n in range(NB):
                w_sb = wpool.tile([P, KT, NF], mybir.dt.bfloat16)
                nc.sync.dma_start(w_sb[:], wt_r[:, :, n * NF:(n + 1) * NF])

                for m in range(MT):
                    pt = psum.tile([P, NF], mybir.dt.float32)
                    for k in range(KT):
                        nc.tensor.matmul(
                            pt[:],
                            xt_sb[:, k, m * P:(m + 1) * P],
                            w_sb[:, k, :],
                            start=(k == 0),
                            stop=(k == KT - 1),
                        )
                    ot = opool.tile([P, NF], mybir.dt.float32)
                    nc.vector.tensor_add(
                        ot[:], pt[:], bias_sb[:, n * NF:(n + 1) * NF])
                    nc.sync.dma_start(
                        y_ap[m * P:(m + 1) * P, n * NF:(n + 1) * NF], ot[:])

    nc.compile()
    return nc


def _prep_inputs(x, weight, bias):
    x = np.asarray(x, dtype=np.float32)
    weight = np.asarray(weight, dtype=np.float32)
    bias = np.asarray(bias, dtype=np.float32)

    wt = np.ascontiguousarray(np.sign(weight).T).astype(ml_dtypes.bfloat16)
    xt = np.ascontiguousarray(x.transpose(0, 2, 1)).astype(ml_dtypes.bfloat16)
    bias_bc = np.ascontiguousarray(np.broadcast_to(bias[None, :], (P, DOUT)))
    return xt, wt, bias_bc


def kernel(x, weight, bias, _trace=False):
    xt, wt, bias_bc = _prep_inputs(x, weight, bias)

    nc = build_nc()
    core_ids = list(range(N_CORES))
    in_maps = [{"xt": xt[c], "wt": wt, "bias": bias_bc} for c in core_ids]
    res = run_bass_kernel_spmd(nc, in_maps, core_ids, trace=_trace)

    out = np.empty((B, S, DOUT), dtype=np.float32)
    for c in core_ids:
        out[c] = res.results[c]["y"]
    if _trace:
        kernel.last_result = res
    return out


# revision 3
# speedup vs baseline: 1.0498x; 1.0498x over previous
"""BinaryLinear kernel for Trainium2, 8 NeuronCores.

y = x @ sign(W)^T + bias
  x: (8, 1024, 4096) f32, W: (4096, 4096) f32, bias: (4096,) f32
  y: (8, 1024, 4096) f32

Strategy: data-parallel over the batch dim (8 batches -> 8 cores).
Each core computes y_c[1024, 4096] = x_c @ sign(W)^T + bias as a bf16
matmul (sign(W) is exactly representable in bf16) with fp32 PSUM
accumulation. Host-side we only reshape/cast: wt = sign(W).T and
xt = x_c.T so the contraction dim lands on SBUF partitions with
contiguous DMA. Inputs are DMA'd in chunks (per-m x tiles, per-k w
chunks) so the tensor engine starts within a few us of kernel start.
"""

import numpy as np
import ml_dtypes

import concourse.bass as bass
import concourse.tile as tile
from concourse import bacc, mybir
from concourse.bass_utils import run_bass_kernel_spmd

# Problem shapes (hardcoded per contract)
B, S, DIN, DOUT = 8, 1024, 4096, 4096
P = 128            # SBUF partitions / contraction tile
KT = DIN // P      # 32 contraction tiles
MT = S // P        # 8 row tiles of output (s dim)
NF = 512           # matmul moving free dim / PSUM bank width (fp32)
NB = DOUT // NF    # 8 column blocks of output (o dim)
KC = 4             # w-block k-chunks per n block
KSUB = KT // KC    # 8 k tiles per chunk

N_CORES = 8


def build_nc():
    nc = bacc.Bacc("TRN2", target_bir_lowering=False, debug=False,
                   num_devices=N_CORES)
    # xt: x_c.T tiled host-side as [m, p, k, j] so each per-m DMA reads
    # contiguous 8 KiB runs per partition.
    xt = nc.dram_tensor("xt", [MT, P, KT, P], mybir.dt.bfloat16,
                        kind="ExternalInput")
    wt = nc.dram_tensor("wt", [DIN, DOUT], mybir.dt.bfloat16,
                        kind="ExternalInput")
    bias = nc.dram_tensor("bias", [P, DOUT], mybir.dt.float32,
                          kind="ExternalInput")
    y = nc.dram_tensor("y", [S, DOUT], mybir.dt.float32, kind="ExternalOutput")

    xt_ap = xt.ap()
    wt_r = wt.ap().rearrange("(k p) o -> p k o", p=P)     # [128, 32, 4096]
    y_ap = y.ap()
    bias_ap = bias.ap()

    with tile.TileContext(nc) as tc:
        with (
            tc.tile_pool(name="xpool", bufs=1) as xpool,
            tc.tile_pool(name="bpool", bufs=1) as bpool,
            tc.tile_pool(name="wpool", bufs=2) as wpool,
            tc.tile_pool(name="opool", bufs=4) as opool,
            tc.tile_pool(name="psum", bufs=6, space="PSUM") as psum,
        ):
            def load_w_chunks(n):
                chunks = []
                for c in range(KC):
                    w_sb = wpool.tile([P, KSUB, NF], mybir.dt.bfloat16,
                                      name=f"w_{c}", tag=f"w_{c}")
                    nc.sync.dma_start(
                        w_sb[:],
                        wt_r[:, c * KSUB:(c + 1) * KSUB, n * NF:(n + 1) * NF])
                    chunks.append(w_sb)
                return chunks

            # Prologue: first-needed data first (m=0 x tile, n=0 w chunks),
            # then bias, then the rest of x.
            xt_tiles = []
            xt_tiles.append(xpool.tile([P, KT, P], mybir.dt.bfloat16,
                                       name="xt_0", tag="xt_0"))
            nc.sync.dma_start(xt_tiles[0][:], xt_ap[0])

            w_chunks = load_w_chunks(0)

            bias_sb = bpool.tile([P, DOUT], mybir.dt.float32)
            nc.sync.dma_start(bias_sb[:], bias_ap[:])

            for m in range(1, MT):
                t = xpool.tile([P, KT, P], mybir.dt.bfloat16, name=f"xt_{m}", tag=f"xt_{m}")
                nc.sync.dma_start(t[:], xt_ap[m])
                xt_tiles.append(t)

            for n in range(NB):
                for m in range(MT):
                    pt = psum.tile([P, NF], mybir.dt.float32)
                    for k in range(KT):
                        nc.tensor.matmul(
                            pt[:],
                            xt_tiles[m][:, k, :],
                            w_chunks[k // KSUB][:, k % KSUB, :],
                            start=(k == 0),
                            stop=(k == KT - 1),
                        )
                    if m == 0 and n + 1 < NB:
                        next_chunks = load_w_chunks(n + 1)
                    ot = opool.tile([P, NF], mybir.dt.float32)
                    nc.vector.tensor_add(
                        ot[:], pt[:], bias_sb[:, n * NF:(n + 1) * NF])
                    nc.sync.dma_start(
                        y_ap[m * P:(m + 1) * P, n * NF:(n + 1) * NF], ot[:])
                if n + 1 < NB:
                    w_chunks = next_chunks

    nc.compile()
    return nc


def _prep_inputs(x, weight, bias):
    x = np.asarray(x, dtype=np.float32)
    weight = np.asarray(weight, dtype=np.float32)
    bias = np.asarray(bias, dtype=np.float32)

    wt = np.ascontiguousarray(np.sign(weight).T).astype(ml_dtypes.bfloat16)
    # [b, s, i] -> per-core [m, p(i%128), k(i//128), j(s%128)]
    xb = x.astype(ml_dtypes.bfloat16)
    xt = np.ascontiguousarray(
        xb.reshape(B, MT, P, KT, P).transpose(0, 1, 4, 3, 2))
    bias_bc = np.ascontiguousarray(np.broadcast_to(bias[None, :], (P, DOUT)))
    return xt, wt, bias_bc


def kernel(x, weight, bias, _trace=False):
    xt, wt, bias_bc = _prep_inputs(x, weight, bias)

    nc = build_nc()
    core_ids = list(range(N_CORES))
    in_maps = [{"xt": xt[c], "wt": wt, "bias": bias_bc} for c in core_ids]
    res = run_bass_kernel_spmd(nc, in_maps, core_ids, trace=_trace)

    out = np.empty((B, S, DOUT), dtype=np.float32)
    for c in core_ids:
        out[c] = res.results[c]["y"]
    if _trace:
        kernel.last_result = res
    return out
